# revision 29
# baseline (speedup 1.0000x reference)
"""Trainium2 Bass kernel for LlamaAttention (B=1, S=2048, HID=2048, H=32, KV=8, D=64).

Sharding (8 cores): tensor-parallel over heads. Core c owns q-heads 4c..4c+3 and
kv-head c. Each core computes QKV projections for its heads, RoPE, causal
attention; attention outputs are normalized per 512-query tile, AllGathered in
bf16 chunks overlapped with the next tile's attention compute, and each core
computes 256 output features of o_proj (Wo row-sharded). Host concatenates the
8 column shards and transposes.

Matmuls run in bf16 (fp32 PSUM accumulation); softmax/rope arithmetic in fp32.
"""

import numpy as np
import ml_dtypes

import concourse.bass as bass
import concourse.mybir as mybir
import concourse.tile as tile
from concourse import bacc
from concourse import bass_utils
from concourse.bass_interp import get_hw_module
from concourse.masks import make_identity

S = 2048
HID = 2048
H = 32
KV = 8
D = 64
NCORES = 8
HQ = H // NCORES          # 4 q heads per core
BASE = 10000.0
F32 = mybir.dt.float32
BF16 = mybir.dt.bfloat16
AF = mybir.ActivationFunctionType
ST = S // 512             # 4 s/q tiles of 512
KO = HID // 128           # 16 contraction chunks
NEG = -1.0e30
HALF_PI = float(np.pi / 2)
LN_BASE = float(np.log(BASE))
TWO_PI = 6.283185307179586


def build_body(tc, aps):
    nc = tc.nc
    hiddenT = aps["hiddenT"]
    wqkvT = aps["wqkvT"]
    woT = aps["woT"]
    trimask = aps["trimask"]
    outT = aps["outT"]

    hT3 = hiddenT.rearrange("(ko p) s -> p ko s", p=128)
    wq3 = wqkvT.rearrange("(ko p) m -> p ko m", p=128)
    wo3 = woT.rearrange("(ko p) m -> p ko m", p=128)

    from contextlib import ExitStack
    es = ExitStack()
    const_pool = es.enter_context(tc.tile_pool(name="const", bufs=1))
    qkv_pool = es.enter_context(tc.tile_pool(name="qkvout", bufs=1))
    tmp_pool = es.enter_context(tc.tile_pool(name="ropetmp", bufs=3))
    dram = es.enter_context(tc.tile_pool(name="dram", bufs=1, space="DRAM"))

    # ---- weights + constants (Sync DMA queue; ht0 follows right behind) ----
    # first contraction chunks of the weights land first so the very first
    # matmul can issue while the rest of the input stream is in flight
    wq_sb = qkv_pool.tile([128, KO, 384], BF16, tag="wq")
    nc.sync.dma_start(wq_sb[:, 0:4, :], wq3[:, 0:4, :])
    mask_sb = const_pool.tile([128, 128], F32, tag="mask")
    nc.sync.dma_start(mask_sb[:], trimask[:])
    ident = const_pool.tile([64, 64], BF16, tag="ident")
    make_identity(nc, ident[:])

    # ---- RoPE table constants ----
    cosT2 = const_pool.tile([128, S], F32, tag="cos")
    sinT2 = const_pool.tile([128, S], F32, tag="sin")
    posi = aps["posi"]
    powers = aps["powers"]
    hpi = const_pool.tile([128, 1], F32, tag="hpi")
    nc.gpsimd.memset(hpi[:], HALF_PI)

    def tables_setup():
        # inv_freq = exp(-ln(BASE) * sigmoid(powers))  [32, 1]
        pw = const_pool.tile([32, 1], F32, tag="pw")
        nc.sync.dma_start(pw[:], powers[:])
        sg = const_pool.tile([32, 1], F32, tag="sg")
        nc.scalar.activation(sg[:], pw[:], AF.Sigmoid)
        invf = const_pool.tile([32, 1], F32, tag="invf")
        nc.scalar.activation(invf[:], sg[:], AF.Exp, scale=-LN_BASE)
        # signed replicate to [128, 1]: bands (-f, +f, -f, +f)
        invs = const_pool.tile([128, 1], F32, tag="invs")
        for b in range(4):
            nc.sync.dma_start(invs[b * 32:(b + 1) * 32, :], invf[:])
        for b in (0, 2):
            nc.scalar.activation(
                invs[b * 32:(b + 1) * 32, :], invs[b * 32:(b + 1) * 32, :],
                AF.Copy, scale=-1.0)
        pos_i = const_pool.tile([1, S], mybir.dt.int32, tag="tposi")
        nc.sync.dma_start(pos_i[:], posi[:])
        return invs, pos_i

    def tables_chunk(st, invs, pos_i):
        # freqs = pos*invf_signed, range-reduce mod 2pi, sin/cos
        sl = slice(st * 512, (st + 1) * 512)
        pbi = tmp_pool.tile([128, 512], mybir.dt.int32, tag="tpb")
        nc.gpsimd.partition_broadcast(pbi[:], pos_i[0:1, sl])
        pb = tmp_pool.tile([128, 512], F32, tag="tpf")
        nc.vector.tensor_copy(pb[:], pbi[:])
        nc.vector.tensor_scalar_mul(cosT2[:, sl], pb[:], invs[:, 0:1])
        u = tmp_pool.tile([128, 512], F32, tag="tu")
        nc.vector.tensor_scalar_mul(u[:], cosT2[:, sl], 1.0 / TWO_PI)
        ui = tmp_pool.tile([128, 512], mybir.dt.int32, tag="tui")
        nc.vector.tensor_copy(ui[:], u[:])
        nc.vector.tensor_copy(u[:], ui[:])
        nc.vector.tensor_scalar_mul(u[:], u[:], -TWO_PI)
        nc.vector.tensor_add(cosT2[:, sl], cosT2[:, sl], u[:])
        nc.scalar.activation(sinT2[:, sl], cosT2[:, sl], AF.Sin)
        nc.scalar.activation(cosT2[:, sl], cosT2[:, sl], AF.Sin, bias=hpi[:])

    # ---- persistent QKV outputs ----
    qT = [qkv_pool.tile([128, S], BF16, tag=f"qT{p}", name=f"qT{p}") for p in range(2)]
    kT2 = qkv_pool.tile([128, S], BF16, tag="kT2")
    vT = qkv_pool.tile([64, S], BF16, tag="vT")
    vones = qkv_pool.tile([128, KO, 65], BF16, tag="vones")
    ones_f = const_pool.tile([128, 1], BF16, tag="onesf")
    nc.gpsimd.memset(ones_f[:], 1.0)
    nc.vector.tensor_copy(vones[:, :, 64:65], ones_f[:, 0:1, None].to_broadcast((128, KO, 1)))

    # ---- QKV projection + RoPE ----
    def rope(ps, dst, n_half, st):
        """ps: psum [64*n_half, 512] raw. dst[:, st*512:...] = roped (bf16)."""
        sl = slice(st * 512, (st + 1) * 512)
        cs = cosT2[0:64 * n_half, sl]
        sn = sinT2[0:64 * n_half, sl]
        craw = tmp_pool.tile([64 * n_half, 512], F32, tag="craw")
        nc.scalar.activation(craw[:], ps, AF.Copy)
        sw = tmp_pool.tile([64 * n_half, 512], F32, tag="swap")
        for b in range(n_half):
            nc.sync.dma_start(sw[b * 64:b * 64 + 32, :], craw[b * 64 + 32:b * 64 + 64, :])
            nc.sync.dma_start(sw[b * 64 + 32:b * 64 + 64, :], craw[b * 64:b * 64 + 32, :])
        t1 = tmp_pool.tile([64 * n_half, 512], F32, tag="t1")
        nc.vector.tensor_mul(t1[:], ps, cs)
        nc.vector.tensor_mul(sw[:], sw[:], sn)
        nc.vector.tensor_add(dst[0:64 * n_half, sl], t1[:], sw[:])

    with (
        tc.tile_pool(name="hidd", bufs=2) as hidd_pool,
        tc.tile_pool(name="qkvps", bufs=3, space="PSUM") as qkv_ps,
        tc.tile_pool(name="kvps", bufs=2, space="PSUM") as kv_ps,
        tc.tile_pool(name="vtps", bufs=2, space="PSUM") as vt_ps,
    ):
        # prefetch the first two hidden-state tiles before anything else can
        # claim the Sync DMA queue, then interleave the RoPE table chunks
        # with the projection matmuls
        ht_pre = []
        for st in range(2):
            ht = hidd_pool.tile([128, KO, 512], BF16, tag="ht",
                                name=f"ht_pre{st}")
            if st == 0:
                nc.sync.dma_start(ht[:, 0:4, :], hT3[:, 0:4, 0:512])
                nc.sync.dma_start(wq_sb[:, 4:KO, :], wq3[:, 4:KO, :])
                nc.sync.dma_start(ht[:, 4:KO, :], hT3[:, 4:KO, 0:512])
            else:
                nc.sync.dma_start(ht[:], hT3[:, :, st * 512:(st + 1) * 512])
            ht_pre.append(ht)
        invs, pos_i = tables_setup()
        for st in range(ST):
            tables_chunk(st, invs, pos_i)
            if st < 2:
                ht = ht_pre[st]
            else:
                ht = hidd_pool.tile([128, KO, 512], BF16, tag="ht")
                nc.sync.dma_start(ht[:], hT3[:, :, st * 512:(st + 1) * 512])
            for mt in range(2):  # q head pairs
                ps = qkv_ps.tile([128, 512], F32, tag="qps")
                for ko in range(KO):
                    nc.tensor.matmul(
                        ps[:], wq_sb[:, ko, mt * 128:(mt + 1) * 128],
                        ht[:, ko, :], start=(ko == 0), stop=(ko == KO - 1))
                rope(ps[:], qT[mt], 2, st)
            # k+v combined (M=128: rows 0-63 = k, 64-127 = v)
            pskv = kv_ps.tile([128, 512], F32, tag="kvp")
            for ko in range(KO):
                nc.tensor.matmul(pskv[:], wq_sb[:, ko, 256:384], ht[:, ko, :],
                                 start=(ko == 0), stop=(ko == KO - 1))
            rope(pskv[0:64, :], kT2, 1, st)
            nc.sync.dma_start(kT2[64:128, st * 512:(st + 1) * 512],
                              kT2[0:64, st * 512:(st + 1) * 512])
            nc.scalar.activation(vT[:, st * 512:(st + 1) * 512], pskv[64:128, :],
                                 AF.Copy)
            # transpose v chunks of this s-tile into vones [128, ki, 0:64]
            for kl in range(4):
                ki = st * 4 + kl
                pvt = vt_ps.tile([128, 64], BF16, tag="vt")
                nc.tensor.transpose(pvt[:], vT[:, ki * 128:(ki + 1) * 128], ident[:])
                nc.vector.tensor_copy(vones[:, ki, 0:64], pvt[:])

    # o_proj weights: only needed from oproj(0) onwards — load late so the
    # descriptor gen never delays the QKV input stream
    wo_sb = qkv_pool.tile([128, KO, 256], BF16, tag="wo")
    nc.sync.dma_start(wo_sb[:], wo3)

    # ---- attention + overlapped AllGather + o_proj ----
    cc_in = [dram.tile([HQ * D, 512], BF16, tag=f"cc_in{qt}", name=f"cc_in{qt}")
             for qt in range(ST)]
    cc_out = [dram.tile([H * D, 512], BF16, tag=f"cc_out{qt}", name=f"cc_out{qt}")
              for qt in range(ST)]

    with (
        tc.tile_pool(name="sps", bufs=4, space="PSUM") as s_ps,
        tc.tile_pool(name="aps", bufs=2, space="PSUM") as a_ps,
        tc.tile_pool(name="ops", bufs=2, space="PSUM") as o_ps,
        tc.tile_pool(name="expp", bufs=30) as exp_pool,
        tc.tile_pool(name="norm", bufs=4) as norm_pool,
        tc.tile_pool(name="attf", bufs=2) as attf_pool,
        tc.tile_pool(name="oout", bufs=2) as oout_pool,
    ):
        def oproj(qt):
            co3 = cc_out[qt].rearrange("(ko p) s -> p ko s", p=128)
            af = attf_pool.tile([128, KO, 512], BF16, tag="af")
            nc.sync.dma_start(af[:], co3)
            for ft in range(2):
                po = o_ps.tile([128, 512], F32, tag="po")
                for ko in range(KO):
                    nc.tensor.matmul(po[:], wo_sb[:, ko, ft * 128:(ft + 1) * 128],
                                     af[:, ko, :], start=(ko == 0),
                                     stop=(ko == KO - 1))
                ot = oout_pool.tile([128, 512], F32, tag="ot")
                nc.vector.tensor_copy(ot[:], po[:])
                nc.sync.dma_start(
                    outT[ft * 128:(ft + 1) * 128, qt * 512:(qt + 1) * 512], ot[:])

        # qt=3 first: its AllGather absorbs the cross-core skew while the
        # other three tiles compute, and the final AllGather (qt=2) hits an
        # otherwise-quiet DMA fabric
        qt_order = [3, 0, 1, 2]
        prev_qt = None
        for qt in qt_order:
            nki = 4 * qt + 4
            for hp in range(2):
                pa = [a_ps.tile([65, 512], F32, tag="pattn",
                                name=f"pattn{qt}_{hp}_{i}") for i in range(2)]
                staged = []
                for ki in range(nki):
                    for x in range(2):  # head 2hp+x
                        m = ki - 4 * qt
                        lo = max(0, m) * 128
                        pss = s_ps.tile([128, 512], F32, tag="ps_s")
                        nc.tensor.matmul(
                            pss[:, lo:512],
                            kT2[x * 64:(x + 1) * 64, ki * 128:(ki + 1) * 128],
                            qT[hp][x * 64:(x + 1) * 64, qt * 512 + lo:(qt + 1) * 512],
                            start=True, stop=True)
                        if m >= 0:  # diagonal block: apply triangular causal mask
                            nc.vector.tensor_add(
                                pss[:, m * 128:(m + 1) * 128],
                                pss[:, m * 128:(m + 1) * 128], mask_sb[:])
                        et = exp_pool.tile([128, 512], BF16, tag="expt")
                        nc.scalar.activation(et[:, lo:512], pss[:, lo:512],
                                             AF.Exp, scale=0.125)
                        staged.append((ki, x, lo, et))
                # second pass: attn@v matmuls read staged SBUF exp tiles, so the
                # PE never blocks on ACT latency mid-stream
                for ki, x, lo, et in staged:
                    nc.tensor.matmul(pa[x][:, lo:512], vones[:, ki, :],
                                     et[:, lo:512],
                                     start=(ki == 0), stop=(ki == nki - 1))
                # pull the finished heads out of PSUM immediately (DVE only) so
                # the next head-pair's PSUM never waits on the normalize chain
                un = [norm_pool.tile([65, 512], F32, tag="un",
                                     name=f"un{qt}_{hp}_{i}") for i in range(2)]
                for x in range(2):
                    nc.vector.tensor_copy(un[x][:], pa[x][:])
                # softmax normalize: r = 1/sums, per-query scale, bf16 cast
                rp = norm_pool.tile([128, 8], F32, tag="rp")
                for x in range(2):
                    nc.sync.dma_start(
                        rp[x * 64:(x + 1) * 64, :],
                        un[x][64:65, :].rearrange("q (p f) -> q p f", p=64))
                nc.vector.reciprocal_approx_fast(rp[:], rp[:])
                r2 = norm_pool.tile([1, 2 * 512], F32, tag="r2")
                nc.sync.dma_start(r2[0:1, :].rearrange(
                    "q (p f) -> q p f", p=128), rp[:])
                for x in range(2):
                    rbc = norm_pool.tile([64, 512], F32, tag="rbc")
                    nc.gpsimd.partition_broadcast(
                        rbc[:], r2[:, x * 512:(x + 1) * 512])
                    fin = norm_pool.tile([64, 512], BF16, tag="fin")
                    nc.vector.tensor_mul(fin[:], un[x][0:64, :], rbc[:])
                    h = 2 * hp + x
                    nc.sync.dma_start(cc_in[qt][h * 64:(h + 1) * 64, :], fin[:])
            nc.gpsimd.collective_compute(
                "AllGather", mybir.AluOpType.bypass,
                ins=[cc_in[qt].opt()], outs=[cc_out[qt].opt()],
                replica_groups=[list(range(NCORES))],
            )
            if prev_qt is not None:
                oproj(prev_qt)
            prev_qt = qt
        oproj(prev_qt)
    es.close()


_CACHE = {}


def build_program():
    if "nc" in _CACHE:
        return _CACHE["nc"]
    nc = bacc.Bacc("TRN2", target_bir_lowering=False, debug=False,
                   enable_asserts=True, num_devices=NCORES)
    aps = {}
    aps["hiddenT"] = nc.dram_tensor("hiddenT", [HID, S], BF16, kind="ExternalInput").ap()
    aps["wqkvT"] = nc.dram_tensor("wqkvT", [HID, (HQ + 2) * D], BF16, kind="ExternalInput").ap()
    aps["woT"] = nc.dram_tensor("woT", [HID, HQ * D], BF16, kind="ExternalInput").ap()
    aps["trimask"] = nc.dram_tensor("trimask", [128, 128], F32, kind="ExternalInput").ap()
    aps["posi"] = nc.dram_tensor("posi", [1, S], mybir.dt.int32, kind="ExternalInput").ap()
    aps["powers"] = nc.dram_tensor("powers", [D // 2, 1], F32, kind="ExternalInput").ap()
    aps["outT"] = nc.dram_tensor("outT", [HQ * D, S], F32, kind="ExternalOutput").ap()

    with tile.TileContext(nc) as tc:
        build_body(tc, aps)
    nc.compile()
    _CACHE["nc"] = nc
    return nc


def make_in_maps(hidden_states, position_ids, powers, Wq, Wk, Wv, Wo):
    bf16 = ml_dtypes.bfloat16
    hidden = np.asarray(hidden_states, np.float32).reshape(S, HID)
    hiddenT = np.ascontiguousarray(hidden.T).astype(bf16)
    pos = np.asarray(position_ids, np.int32).reshape(1, S)
    pw = np.asarray(powers, np.float32).reshape(D // 2, 1)
    Wq = np.asarray(Wq, np.float32)
    Wk = np.asarray(Wk, np.float32)
    Wv = np.asarray(Wv, np.float32)
    Wo = np.asarray(Wo, np.float32)
    kl = np.arange(128)[:, None]
    ql = np.arange(128)[None, :]
    trimask = np.where(kl <= ql, 0.0, NEG).astype(np.float32)

    in_maps = []
    for c in range(NCORES):
        wqkv = np.concatenate([
            Wq[c * HQ * D:(c + 1) * HQ * D],          # [256, HID]
            Wk[c * D:(c + 1) * D],                    # [64, HID]
            Wv[c * D:(c + 1) * D],                    # [64, HID]
        ], axis=0)                                    # [384, HID]
        m = {
            "hiddenT": hiddenT,
            "wqkvT": np.ascontiguousarray(wqkv.T).astype(bf16),
            "woT": np.ascontiguousarray(Wo[c * HQ * D:(c + 1) * HQ * D].T).astype(bf16),
            "trimask": trimask,
            "posi": pos,
            "powers": pw,
        }
        in_maps.append(m)
    return in_maps


def run_spmd(nc, in_maps, **kwargs):
    m = nc.m
    nc.m = get_hw_module(nc.m)
    try:
        return bass_utils.run_bass_kernel_spmd(
            nc, in_maps, core_ids=list(range(NCORES)), **kwargs)
    finally:
        nc.m = m


def kernel(hidden_states, position_ids, powers, Wq, Wk, Wv, Wo):
    nc = build_program()
    in_maps = make_in_maps(hidden_states, position_ids, powers, Wq, Wk, Wv, Wo)
    res = run_spmd(nc, in_maps)
    outT_full = np.concatenate([res.results[c]["outT"] for c in range(NCORES)], axis=0)
    return np.ascontiguousarray(outT_full.T).reshape(1, S, HID).astype(np.float32)


if __name__ == "__main__":
    rng = np.random.default_rng(0)
    inputs = {
        "hidden_states": rng.standard_normal((1, S, HID), dtype=np.float32),
        "position_ids": np.broadcast_to(np.arange(S, dtype=np.int32), (1, S)),
        "powers": rng.standard_normal(D // 2).astype(np.float32),
        "Wq": (rng.standard_normal((H * D, HID)) * 0.02).astype(np.float32),
        "Wk": (rng.standard_normal((KV * D, HID)) * 0.02).astype(np.float32),
        "Wv": (rng.standard_normal((KV * D, HID)) * 0.02).astype(np.float32),
        "Wo": (rng.standard_normal((HID, H * D)) * 0.02).astype(np.float32),
    }
    out = kernel(**inputs)
    print("out", out.shape, out.dtype, np.abs(out).max())
